# revision 28
# baseline (speedup 1.0000x reference)
"""Multi-head self-attention (B=2, T=2048, C=1024, H=16) on 8 NeuronCores.

Sharding: core c -> (batch b = c//4, head-group g = c%4); each core computes
4 heads' attention for one batch plus its slice of the QKV/out projections.
Per-core partial outputs (over head groups) are summed on the host.

Device-side layout is fully transposed (feature dim on partitions):
  xt [C, T] -> QT/KT [256, T] (j on partitions), V natural [T, 256],
  ST = K Qt (scores transposed, tk on partitions).
The stationary PV operand is V extended with 64 columns of ones, so the
yext accumulator's rows 0..63 all hold the softmax denominator — a free
hardware broadcast; normalization = reciprocal_approx_fast + multiply on DVE.

Schedule: the ScalarE exp stream (128 x ~1.07us, unsplittable to any other
engine) is the kernel's critical path. Emission order starts the first
score tile ~12us in (x DMA'd in 4 T-chunks, K proj per chunk) and threads
all remaining PE work (V proj, K jt1, Q blocks, out-projection) into the
exp-paced attention stream as fillers, so PE slack hides under exp.
Iteration order is qb-major; out-projection for q-block qb is emitted
during qb+1, output stored fp16 to halve the store traffic.
"""

import numpy as np

import concourse.bacc as bacc
import concourse.mybir as mybir
import concourse.tile as tile
from concourse.bass_utils import run_bass_kernel_spmd

B, T, C, H = 2, 2048, 1024, 16
HD = C // H  # 64
NCORES = 8
GROUPS = 4  # head groups (one per core within a batch)
HPG = H // GROUPS  # heads per group = 4
JW = HPG * HD  # per-core projection slice width = 256

F32 = mybir.dt.float32
MMDT = mybir.dt.float16
NPDT = np.float16

_CACHED_NC = None


def _build(debug=False):
    nc = bacc.Bacc("TRN2", target_bir_lowering=False, num_devices=NCORES)

    xt = nc.dram_tensor("xt", [C, T], MMDT, kind="ExternalInput")
    wq = nc.dram_tensor("wq", [C, JW], MMDT, kind="ExternalInput")
    wk = nc.dram_tensor("wk", [C, JW], MMDT, kind="ExternalInput")
    wv = nc.dram_tensor("wv", [C, JW], MMDT, kind="ExternalInput")
    wo = nc.dram_tensor("wo", [JW, C], MMDT, kind="ExternalInput")
    bq = nc.dram_tensor("bq", [JW], F32, kind="ExternalInput")
    bk = nc.dram_tensor("bk", [JW], F32, kind="ExternalInput")
    out = nc.dram_tensor("out", [T, C], MMDT, kind="ExternalOutput")

    xt_ap = xt[:, :].rearrange("(cc p) t -> p cc t", p=128)  # [128, 8, T]
    wq_ap = wq[:, :].rearrange("(cc p) j -> p cc j", p=128)  # [128, 8, 256]
    wk_ap = wk[:, :].rearrange("(cc p) j -> p cc j", p=128)
    wv_ap = wv[:, :].rearrange("(cc p) j -> p cc j", p=128)
    wo_ap = wo[:, :].rearrange("(jt p) m -> p jt m", p=128)  # [128, 2, C]
    bq_ap = bq[:].rearrange("(jt p) -> p jt", p=128)  # [128, 2]
    bk_ap = bk[:].rearrange("(jt p) -> p jt", p=128)

    with tile.TileContext(nc) as tc:
        with (
            tc.tile_pool(name="big", bufs=1) as big,
            tc.tile_pool(name="work", bufs=16) as work,
            tc.tile_pool(name="nrm", bufs=4) as nrm,
            tc.tile_pool(name="outp", bufs=4) as outp,
            tc.tile_pool(name="psA", bufs=2, space="PSUM") as psA,
            tc.tile_pool(name="psY", bufs=2, space="PSUM") as psY,
        ):
            # ---- persistent SBUF tensors ----
            xt_sb = big.tile([128, 8, T], MMDT)
            wq_sb = big.tile([128, 8, JW], MMDT)
            wk_sb = big.tile([128, 8, JW], MMDT)
            wv_sb = big.tile([128, 8, JW], MMDT)
            wo_sb = big.tile([128, 2, C], MMDT)
            qt_sb = big.tile([128, 2, T], MMDT)
            kt_sb = big.tile([128, 2, T], MMDT)
            yt_sb = big.tile([128, 2, T], MMDT)
            # V natural + 64 ones columns per head (denominator broadcast rows)
            v_sb = big.tile([128, 16, HPG, 128], MMDT)
            bq_sb = big.tile([128, 2], F32)
            bk_sb = big.tile([128, 2], F32)

            # DMA: ordered by first use. x split into 4 T-chunks so K proj
            # (and the first score tiles) start long before x fully lands.
            nc.sync.dma_start(out=wk_sb[:], in_=wk_ap)
            nc.sync.dma_start(out=xt_sb[:, :, 0:512], in_=xt_ap[:, :, 0:512])
            nc.sync.dma_start(out=wq_sb[:], in_=wq_ap)
            nc.sync.dma_start(out=bk_sb[:], in_=bk_ap)
            nc.sync.dma_start(out=bq_sb[:], in_=bq_ap)
            nc.sync.dma_start(out=wv_sb[:], in_=wv_ap)
            for tcn in range(1, 4):
                ts = slice(tcn * 512, (tcn + 1) * 512)
                nc.sync.dma_start(out=xt_sb[:, :, ts], in_=xt_ap[:, :, ts])
            nc.sync.dma_start(out=wo_sb[:], in_=wo_ap)
            nc.vector.memset(v_sb[:, :, :, 0:HD], 1.0)

            def _proj_chunk(w_sb, b_sb, o_sb, jt, tcn):
                """QT/KT for feature tile jt, T-chunk tcn (512 cols)."""
                ts = slice(tcn * 512, (tcn + 1) * 512)
                p1 = psA.tile([128, 512], F32, tag="mm", name="p1")
                for cc in range(8):
                    nc.tensor.matmul(
                        p1[:],
                        w_sb[:, cc, jt * 128 : (jt + 1) * 128],
                        xt_sb[:, cc, ts],
                        start=(cc == 0),
                        stop=(cc == 7),
                    )
                nc.vector.tensor_scalar_add(
                    out=o_sb[:, jt, ts], in0=p1[:], scalar1=b_sb[:, jt : jt + 1]
                )

            def _k(jt, tcn):
                _proj_chunk(wk_sb, bk_sb, kt_sb, jt, tcn)

            def _q(jt, qb):
                _proj_chunk(wq_sb, bq_sb, qt_sb, jt, qb)

            def _v_chunk_tt(tt):
                pv = psA.tile([128, 512], F32, tag="mm", name="pv")
                for cc in range(8):
                    nc.tensor.matmul(
                        pv[:, 0:JW],
                        xt_sb[:, cc, tt * 128 : (tt + 1) * 128],
                        wv_sb[:, cc, :],
                        start=(cc == 0),
                        stop=(cc == 7),
                    )
                nc.vector.tensor_copy(
                    out=v_sb[:, tt, :, HD:128],
                    in_=pv[:, 0:JW].rearrange("p (h d) -> p h d", h=HPG),
                )

            es_tiles = {}
            # kc ranges per score/exp unit: 5 x 3-kc tiles + 1 single
            UNITS = [(0, 3), (3, 6), (6, 9), (9, 12), (12, 15), (15, 16)]

            def _scores_unit(h, qb, u):
                """Score matmuls + one exp tile for unit u of (h, qb)."""
                jt, pb = h // 2, 64 * (h % 2)
                qs = slice(qb * 512, (qb + 1) * 512)
                lo, hi = UNITS[u]
                w = (hi - lo) * 512
                st = psA.tile([128, 1536], F32, tag="mm", name="st")
                for j, kc in enumerate(range(lo, hi)):
                    nc.tensor.matmul(
                        st[:, j * 512 : (j + 1) * 512],
                        kt_sb[pb : pb + HD, jt, kc * 128 : (kc + 1) * 128],
                        qt_sb[pb : pb + HD, jt, qs],
                        start=True,
                        stop=True,
                    )
                es = work.tile([128, 1536], MMDT, tag="es", name="es")
                nc.scalar.activation(
                    out=es[:, 0:w],
                    in_=st[:, 0:w],
                    func=mybir.ActivationFunctionType.Exp,
                )
                es_tiles[(h, qb, u)] = es

            yext_tiles = {}

            def _pv_unit(h, qb, u):
                if u == 0:
                    yext_tiles[(h, qb)] = psY.tile(
                        [128, 512], F32, tag="yext", name="yext"
                    )
                yext = yext_tiles[(h, qb)]
                lo, hi = UNITS[u]
                es = es_tiles.pop((h, qb, u))
                for j, kc in enumerate(range(lo, hi)):
                    nc.tensor.matmul(
                        yext[:],
                        v_sb[:, kc, h, :],
                        es[:, j * 512 : (j + 1) * 512],
                        start=(kc == 0),
                        stop=(kc == 15),
                    )

            def _norm_chunk(h, qb, ck, w):
                jt, pb = h // 2, 64 * (h % 2)
                yext = yext_tiles[(h, qb)]
                cs = slice(ck * w, (ck + 1) * w)
                qs = slice(qb * 512 + ck * w, qb * 512 + (ck + 1) * w)
                r32 = nrm.tile([HD, w], F32, tag="r32", name="r32")
                nc.vector.reciprocal_approx_fast(out=r32[:], in_=yext[0:HD, cs])
                nc.vector.tensor_mul(
                    out=yt_sb[pb : pb + HD, jt, qs], in0=r32[:], in1=yext[HD:128, cs]
                )

            def _norm(h, qb):
                _norm_chunk(h, qb, 0, 512)
                del yext_tiles[(h, qb)]

            def _outproj_psum(tt, po, jt):
                # jt outer: both mb matmuls share the yt stationary (one LDW)
                for mb in range(2):
                    nc.tensor.matmul(
                        po[:, mb * 512 : (mb + 1) * 512],
                        yt_sb[:, jt, tt * 128 : (tt + 1) * 128],
                        wo_sb[:, jt, mb * 512 : (mb + 1) * 512],
                        start=(jt == 0),
                        stop=(jt == 1),
                    )

            def _outproj_store(tt, po):
                ob = outp.tile([128, 1024], MMDT, tag="ob", name="ob")
                nc.vector.tensor_copy(out=ob[:], in_=po[:])
                nc.sync.dma_start(out=out[tt * 128 : (tt + 1) * 128, :], in_=ob[:])

            op_state = {}

            def _op_unit(tt, part):
                """Out-projection for t-tile tt, split in 2 units of 2 matmuls."""
                if part == 0:
                    op_state[tt] = psA.tile([128, 1024], F32, tag="mm", name="po")
                    _outproj_psum(tt, op_state[tt], 0)
                else:
                    po = op_state.pop(tt)
                    _outproj_psum(tt, po, 1)
                    _outproj_store(tt, po)

            q_state = {}

            def _q_unit(jt, qb, part):
                """Q proj block (8 cc matmuls), split in 4 units of 2."""
                ts = slice(qb * 512, (qb + 1) * 512)
                if part == 0:
                    q_state[(jt, qb)] = psA.tile([128, 512], F32, tag="mm", name="p1")
                p1 = q_state[(jt, qb)]
                for cc in (2 * part, 2 * part + 1):
                    nc.tensor.matmul(
                        p1[:],
                        wq_sb[:, cc, jt * 128 : (jt + 1) * 128],
                        xt_sb[:, cc, ts],
                        start=(cc == 0),
                        stop=(cc == 7),
                    )
                if part == 3:
                    nc.vector.tensor_scalar_add(
                        out=qt_sb[:, jt, ts], in0=p1[:], scalar1=bq_sb[:, jt : jt + 1]
                    )
                    del q_state[(jt, qb)]

            def _outproj(tts):
                # tail variant: whole tiles
                for tt in tts:
                    po = psA.tile([128, 1024], F32, tag="mm", name="po")
                    _outproj_psum(tt, po, 0)
                    _outproj_psum(tt, po, 1)
                    _outproj_store(tt, po)

            # ---- lead-in ----
            # iteration i = (qb, h), qb-major: i = 4*qb + h, jt = h//2.
            # Score unit u covers kc range UNITS[u]; K chunk tc covers kc
            # 4tc..4tc+3, so unit u needs K chunks through (UNITS[u][1]-1)//4.
            _k(0, 0)
            _q(0, 0)
            _scores_unit(0, 0, 0)
            _k(0, 1)
            _scores_unit(0, 0, 1)
            _k(0, 2)
            _scores_unit(0, 0, 2)
            _scores_unit(0, 0, 3)
            _k(0, 3)
            _scores_unit(0, 0, 4)
            _scores_unit(0, 0, 5)
            _q(1, 0)
            # iteration 0: V proj woven under the S(1) exp stream (PV starts
            # at lag 2, so V only has to beat PV(0) in step 1).
            for u in range(6):
                _scores_unit(1, 0, u)
                for tt in range(UNITS[u][0], UNITS[u][1]):
                    _v_chunk_tt(tt)
            for tcn in range(4):
                _k(1, tcn)

            # steady state, step i (1..16): per unit slot: PV(i-1,u), one
            # filler unit, S(i+1,u). Filler units are ~2 matmuls each so PE
            # arrives at each score just as ACT frees its PSUM slab.
            # Constraints: Q(jt,qb) fully by step 4qb+2jt-1; outproj(qb)
            # after N(qb,h3) which lands at end of step 4qb+4.
            fillers = {
                2: [lambda p=p: _q_unit(0, 1, p) for p in range(4)],
                3: [lambda p=p: _q_unit(1, 1, p) for p in range(4)],
                4: [lambda p=p: _q_unit(0, 2, p) for p in range(4)],
                5: [lambda t=t, p=p: _op_unit(t, p) for t in (0, 1) for p in (0, 1)],
                6: [lambda t=t, p=p: _op_unit(t, p) for t in (2, 3) for p in (0, 1)],
                7: [lambda p=p: _q_unit(1, 2, p) for p in range(4)],
                8: [lambda p=p: _q_unit(0, 3, p) for p in range(4)],
                9: [lambda t=t, p=p: _op_unit(t, p) for t in (4, 5) for p in (0, 1)],
                10: [lambda t=t, p=p: _op_unit(t, p) for t in (6, 7) for p in (0, 1)],
                11: [lambda p=p: _q_unit(1, 3, p) for p in range(4)],
                13: [lambda t=t, p=p: _op_unit(t, p) for t in (8, 9) for p in (0, 1)],
                14: [
                    lambda t=t, p=p: _op_unit(t, p) for t in (10, 11) for p in (0, 1)
                ],
            }
            iters = [(qb, h) for qb in range(4) for h in range(HPG)]
            for i in range(1, 17):
                pv_i = i - 1  # PV lags the score stream by one extra step
                qbp, hp = iters[pv_i]
                units = fillers.get(i, [])
                for u in range(6):
                    _pv_unit(hp, qbp, u)
                    if u < len(units):
                        units[u]()
                    if i + 1 < 16:
                        qb1, h1 = iters[i + 1]
                        _scores_unit(h1, qb1, u)
                if pv_i < 15:
                    _norm(hp, qbp)
                if len(units) > 6:
                    for f in units[6:]:
                        f()
            # tail: pipeline the last normalize with the last out-projection
            for ck in range(4):
                _norm_chunk(3, 3, ck, 128)
                _outproj([12 + ck])
            del yext_tiles[(3, 3)]

            if debug:
                dq = nc.dram_tensor("dbg_q", [128, 2, T], MMDT, kind="ExternalOutput")
                dk = nc.dram_tensor("dbg_k", [128, 2, T], MMDT, kind="ExternalOutput")
                dy = nc.dram_tensor("dbg_y", [128, 2, T], MMDT, kind="ExternalOutput")
                dv = nc.dram_tensor(
                    "dbg_v", [128, 16, HPG, 128], MMDT, kind="ExternalOutput"
                )
                nc.sync.dma_start(out=dq[:, :, :], in_=qt_sb[:])
                nc.sync.dma_start(out=dk[:, :, :], in_=kt_sb[:])
                nc.sync.dma_start(out=dy[:, :, :], in_=yt_sb[:])
                nc.sync.dma_start(out=dv[:, :, :, :], in_=v_sb[:])

    nc.finalize()
    return nc


def _get_nc():
    global _CACHED_NC
    if _CACHED_NC is None:
        _CACHED_NC = _build()
    return _CACHED_NC


def make_in_maps(x, Wq, bq, Wk, bk, Wv, Wo):
    """Per-core input dicts (host-side sharding + layout + fp16 cast)."""
    xts = [
        np.ascontiguousarray(np.asarray(x[b], np.float32).T).astype(NPDT)
        for b in range(B)
    ]
    wq_f = np.asarray(Wq, np.float32) / 8.0
    wk_f = np.asarray(Wk, np.float32)
    wv_f = np.asarray(Wv, np.float32)
    wo_f = np.asarray(Wo, np.float32)
    bq_f = np.asarray(bq, np.float32) / 8.0
    bk_f = np.asarray(bk, np.float32)
    in_maps = []
    for c in range(NCORES):
        b, g = c // GROUPS, c % GROUPS
        js = slice(g * JW, (g + 1) * JW)
        in_maps.append(
            {
                "xt": xts[b],
                "wq": np.ascontiguousarray(wq_f[:, js]).astype(NPDT),
                "wk": np.ascontiguousarray(wk_f[:, js]).astype(NPDT),
                "wv": np.ascontiguousarray(wv_f[:, js]).astype(NPDT),
                "wo": np.ascontiguousarray(wo_f[js, :]).astype(NPDT),
                "bq": np.ascontiguousarray(bq_f[js]),
                "bk": np.ascontiguousarray(bk_f[js]),
            }
        )
    return in_maps


def combine(results, bias_row):
    """Sum per-core head-group partials and add the host-side bias row."""
    out = np.zeros((B, T, C), np.float32)
    for c in range(NCORES):
        out[c // GROUPS] += results[c]["out"].astype(np.float32)
    out += bias_row
    return out


def kernel(x, Wq, bq, Wk, bk, Wv, bv, Wo, bo):
    nc = _get_nc()
    in_maps = make_in_maps(x, Wq, bq, Wk, bk, Wv, Wo)
    res = run_bass_kernel_spmd(nc, in_maps, core_ids=list(range(NCORES)))
    bias_row = (
        np.asarray(bv, np.float32) @ np.asarray(Wo, np.float32)
        + np.asarray(bo, np.float32)
    ).astype(np.float32)
    return combine(res.results, bias_row)


# revision 29
# speedup vs baseline: 1.0229x; 1.0229x over previous
"""Multi-head self-attention (B=2, T=2048, C=1024, H=16) on 8 NeuronCores.

Sharding: core c -> (batch b = c//4, head-group g = c%4); each core computes
4 heads' attention for one batch plus its slice of the QKV/out projections.
Per-core partial outputs (over head groups) are summed on the host.

Device-side layout is fully transposed (feature dim on partitions):
  xt [C, T] -> QT/KT [256, T] (j on partitions), V natural [T, 256],
  ST = K Qt (scores transposed, tk on partitions).
The stationary PV operand is V extended with 64 columns of ones, so the
yext accumulator's rows 0..63 all hold the softmax denominator — a free
hardware broadcast; normalization = reciprocal_approx_fast + multiply on DVE.

Schedule: the ScalarE exp stream (128 x ~1.07us, unsplittable to any other
engine) is the kernel's critical path. Emission order starts the first
score tile ~12us in (x DMA'd in 4 T-chunks, K proj per chunk) and threads
all remaining PE work (V proj, K jt1, Q blocks, out-projection) into the
exp-paced attention stream as fillers, so PE slack hides under exp.
Iteration order is qb-major; out-projection for q-block qb is emitted
during qb+1, output stored fp16 to halve the store traffic.
"""

import numpy as np

import concourse.bacc as bacc
import concourse.mybir as mybir
import concourse.tile as tile
from concourse.bass_utils import run_bass_kernel_spmd

B, T, C, H = 2, 2048, 1024, 16
HD = C // H  # 64
NCORES = 8
GROUPS = 4  # head groups (one per core within a batch)
HPG = H // GROUPS  # heads per group = 4
JW = HPG * HD  # per-core projection slice width = 256

F32 = mybir.dt.float32
MMDT = mybir.dt.float16
NPDT = np.float16

_CACHED_NC = None


def _build(debug=False):
    nc = bacc.Bacc("TRN2", target_bir_lowering=False, num_devices=NCORES)

    xt = nc.dram_tensor("xt", [C, T], MMDT, kind="ExternalInput")
    wq = nc.dram_tensor("wq", [C, JW], MMDT, kind="ExternalInput")
    wk = nc.dram_tensor("wk", [C, JW], MMDT, kind="ExternalInput")
    wv = nc.dram_tensor("wv", [C, JW], MMDT, kind="ExternalInput")
    wo = nc.dram_tensor("wo", [JW, C], MMDT, kind="ExternalInput")
    bq = nc.dram_tensor("bq", [JW], F32, kind="ExternalInput")
    bk = nc.dram_tensor("bk", [JW], F32, kind="ExternalInput")
    out = nc.dram_tensor("out", [T, C], MMDT, kind="ExternalOutput")

    xt_ap = xt[:, :].rearrange("(cc p) t -> p cc t", p=128)  # [128, 8, T]
    wq_ap = wq[:, :].rearrange("(cc p) j -> p cc j", p=128)  # [128, 8, 256]
    wk_ap = wk[:, :].rearrange("(cc p) j -> p cc j", p=128)
    wv_ap = wv[:, :].rearrange("(cc p) j -> p cc j", p=128)
    wo_ap = wo[:, :].rearrange("(jt p) m -> p jt m", p=128)  # [128, 2, C]
    bq_ap = bq[:].rearrange("(jt p) -> p jt", p=128)  # [128, 2]
    bk_ap = bk[:].rearrange("(jt p) -> p jt", p=128)

    with tile.TileContext(nc) as tc:
        with (
            tc.tile_pool(name="big", bufs=1) as big,
            tc.tile_pool(name="work", bufs=16) as work,
            tc.tile_pool(name="nrm", bufs=4) as nrm,
            tc.tile_pool(name="outp", bufs=4) as outp,
            tc.tile_pool(name="psA", bufs=3, space="PSUM") as psA,
            tc.tile_pool(name="psY", bufs=2, space="PSUM") as psY,
        ):
            # ---- persistent SBUF tensors ----
            xt_sb = big.tile([128, 8, T], MMDT)
            wq_sb = big.tile([128, 8, JW], MMDT)
            wk_sb = big.tile([128, 8, JW], MMDT)
            wv_sb = big.tile([128, 8, JW], MMDT)
            wo_sb = big.tile([128, 2, C], MMDT)
            qt_sb = big.tile([128, 2, T], MMDT)
            kt_sb = big.tile([128, 2, T], MMDT)
            yt_sb = big.tile([128, 2, T], MMDT)
            # V natural + 64 ones columns per head (denominator broadcast rows)
            v_sb = big.tile([128, 16, HPG, 128], MMDT)
            bq_sb = big.tile([128, 2], F32)
            bk_sb = big.tile([128, 2], F32)

            # DMA: ordered by first use. x split into 4 T-chunks so K proj
            # (and the first score tiles) start long before x fully lands.
            nc.sync.dma_start(out=wk_sb[:], in_=wk_ap)
            nc.sync.dma_start(out=xt_sb[:, :, 0:512], in_=xt_ap[:, :, 0:512])
            nc.sync.dma_start(out=wq_sb[:], in_=wq_ap)
            nc.sync.dma_start(out=bk_sb[:], in_=bk_ap)
            nc.sync.dma_start(out=bq_sb[:], in_=bq_ap)
            nc.sync.dma_start(out=wv_sb[:], in_=wv_ap)
            for tcn in range(1, 4):
                ts = slice(tcn * 512, (tcn + 1) * 512)
                nc.sync.dma_start(out=xt_sb[:, :, ts], in_=xt_ap[:, :, ts])
            nc.sync.dma_start(out=wo_sb[:], in_=wo_ap)
            nc.vector.memset(v_sb[:, :, :, 0:HD], 1.0)

            def _proj_chunk(w_sb, b_sb, o_sb, jt, tcn):
                """QT/KT for feature tile jt, T-chunk tcn (512 cols)."""
                ts = slice(tcn * 512, (tcn + 1) * 512)
                p1 = psA.tile([128, 512], F32, tag="mm", name="p1")
                for cc in range(8):
                    nc.tensor.matmul(
                        p1[:],
                        w_sb[:, cc, jt * 128 : (jt + 1) * 128],
                        xt_sb[:, cc, ts],
                        start=(cc == 0),
                        stop=(cc == 7),
                    )
                nc.vector.tensor_scalar_add(
                    out=o_sb[:, jt, ts], in0=p1[:], scalar1=b_sb[:, jt : jt + 1]
                )

            def _k(jt, tcn):
                _proj_chunk(wk_sb, bk_sb, kt_sb, jt, tcn)

            def _q(jt, qb):
                _proj_chunk(wq_sb, bq_sb, qt_sb, jt, qb)

            def _v_chunk_tt(tt):
                pv = psA.tile([128, 512], F32, tag="mm", name="pv")
                for cc in range(8):
                    nc.tensor.matmul(
                        pv[:, 0:JW],
                        xt_sb[:, cc, tt * 128 : (tt + 1) * 128],
                        wv_sb[:, cc, :],
                        start=(cc == 0),
                        stop=(cc == 7),
                    )
                nc.vector.tensor_copy(
                    out=v_sb[:, tt, :, HD:128],
                    in_=pv[:, 0:JW].rearrange("p (h d) -> p h d", h=HPG),
                )

            es_tiles = {}
            # kc ranges per score/exp unit: 8 x 2-kc tiles
            UNITS = [(2 * u, 2 * u + 2) for u in range(8)]

            def _scores_unit(h, qb, u):
                """Score matmuls + one exp tile for unit u of (h, qb)."""
                jt, pb = h // 2, 64 * (h % 2)
                qs = slice(qb * 512, (qb + 1) * 512)
                lo, hi = UNITS[u]
                w = (hi - lo) * 512
                st = psA.tile([128, 1024], F32, tag="mm", name="st")
                for j, kc in enumerate(range(lo, hi)):
                    nc.tensor.matmul(
                        st[:, j * 512 : (j + 1) * 512],
                        kt_sb[pb : pb + HD, jt, kc * 128 : (kc + 1) * 128],
                        qt_sb[pb : pb + HD, jt, qs],
                        start=True,
                        stop=True,
                    )
                es = work.tile([128, 1024], MMDT, tag="es", name="es")
                nc.scalar.activation(
                    out=es[:, 0:w],
                    in_=st[:, 0:w],
                    func=mybir.ActivationFunctionType.Exp,
                )
                es_tiles[(h, qb, u)] = es

            yext_tiles = {}

            def _pv_unit(h, qb, u):
                if u == 0:
                    yext_tiles[(h, qb)] = psY.tile(
                        [128, 512], F32, tag="yext", name="yext"
                    )
                yext = yext_tiles[(h, qb)]
                lo, hi = UNITS[u]
                es = es_tiles.pop((h, qb, u))
                for j, kc in enumerate(range(lo, hi)):
                    nc.tensor.matmul(
                        yext[:],
                        v_sb[:, kc, h, :],
                        es[:, j * 512 : (j + 1) * 512],
                        start=(kc == 0),
                        stop=(kc == 15),
                    )

            def _norm_chunk(h, qb, ck, w):
                jt, pb = h // 2, 64 * (h % 2)
                yext = yext_tiles[(h, qb)]
                cs = slice(ck * w, (ck + 1) * w)
                qs = slice(qb * 512 + ck * w, qb * 512 + (ck + 1) * w)
                r32 = nrm.tile([HD, w], F32, tag="r32", name="r32")
                nc.vector.reciprocal_approx_fast(out=r32[:], in_=yext[0:HD, cs])
                nc.vector.tensor_mul(
                    out=yt_sb[pb : pb + HD, jt, qs], in0=r32[:], in1=yext[HD:128, cs]
                )

            def _norm(h, qb):
                _norm_chunk(h, qb, 0, 512)
                del yext_tiles[(h, qb)]

            def _outproj_psum(tt, po, jt):
                # jt outer: both mb matmuls share the yt stationary (one LDW)
                for mb in range(2):
                    nc.tensor.matmul(
                        po[:, mb * 512 : (mb + 1) * 512],
                        yt_sb[:, jt, tt * 128 : (tt + 1) * 128],
                        wo_sb[:, jt, mb * 512 : (mb + 1) * 512],
                        start=(jt == 0),
                        stop=(jt == 1),
                    )

            def _outproj_store(tt, po):
                ob = outp.tile([128, 1024], MMDT, tag="ob", name="ob")
                nc.vector.tensor_copy(out=ob[:], in_=po[:])
                nc.sync.dma_start(out=out[tt * 128 : (tt + 1) * 128, :], in_=ob[:])

            op_state = {}

            def _op_unit(tt, part):
                """Out-projection for t-tile tt, split in 2 units of 2 matmuls."""
                if part == 0:
                    op_state[tt] = psA.tile([128, 1024], F32, tag="mm", name="po")
                    _outproj_psum(tt, op_state[tt], 0)
                else:
                    po = op_state.pop(tt)
                    _outproj_psum(tt, po, 1)
                    _outproj_store(tt, po)

            q_state = {}

            def _q_unit(jt, qb, part):
                """Q proj block (8 cc matmuls), split in 4 units of 2."""
                ts = slice(qb * 512, (qb + 1) * 512)
                if part == 0:
                    q_state[(jt, qb)] = psA.tile([128, 512], F32, tag="mm", name="p1")
                p1 = q_state[(jt, qb)]
                for cc in (2 * part, 2 * part + 1):
                    nc.tensor.matmul(
                        p1[:],
                        wq_sb[:, cc, jt * 128 : (jt + 1) * 128],
                        xt_sb[:, cc, ts],
                        start=(cc == 0),
                        stop=(cc == 7),
                    )
                if part == 3:
                    nc.vector.tensor_scalar_add(
                        out=qt_sb[:, jt, ts], in0=p1[:], scalar1=bq_sb[:, jt : jt + 1]
                    )
                    del q_state[(jt, qb)]

            def _outproj(tts):
                # tail variant: whole tiles
                for tt in tts:
                    po = psA.tile([128, 1024], F32, tag="mm", name="po")
                    _outproj_psum(tt, po, 0)
                    _outproj_psum(tt, po, 1)
                    _outproj_store(tt, po)

            # ---- lead-in ----
            # iteration i = (qb, h), qb-major: i = 4*qb + h, jt = h//2.
            # Score unit u covers kc range UNITS[u]; K chunk tc covers kc
            # 4tc..4tc+3, so unit u needs K chunks through (UNITS[u][1]-1)//4.
            _k(0, 0)
            _q(0, 0)
            _scores_unit(0, 0, 0)
            _scores_unit(0, 0, 1)
            _k(0, 1)
            _scores_unit(0, 0, 2)
            _scores_unit(0, 0, 3)
            _k(0, 2)
            _scores_unit(0, 0, 4)
            _scores_unit(0, 0, 5)
            _k(0, 3)
            _scores_unit(0, 0, 6)
            _scores_unit(0, 0, 7)
            _q(1, 0)
            # iteration 0: V proj woven under the S(1) exp stream (PV starts
            # at lag 2, so V only has to beat PV(0) in step 1).
            for u in range(8):
                _scores_unit(1, 0, u)
                for tt in range(UNITS[u][0], UNITS[u][1]):
                    _v_chunk_tt(tt)
            for tcn in range(4):
                _k(1, tcn)

            # steady state, step i (1..16): per unit slot: PV(i-1,u), one
            # filler unit, S(i+1,u). Filler units are ~2 matmuls each so PE
            # arrives at each score just as ACT frees its PSUM slab.
            # Constraints: Q(jt,qb) fully by step 4qb+2jt-1; outproj(qb)
            # after N(qb,h3) which lands at end of step 4qb+4.
            fillers = {
                2: [lambda p=p: _q_unit(0, 1, p) for p in range(4)],
                3: [lambda p=p: _q_unit(1, 1, p) for p in range(4)],
                4: [lambda p=p: _q_unit(0, 2, p) for p in range(4)],
                5: [lambda t=t, p=p: _op_unit(t, p) for t in (0, 1) for p in (0, 1)],
                6: [lambda t=t, p=p: _op_unit(t, p) for t in (2, 3) for p in (0, 1)],
                7: [lambda p=p: _q_unit(1, 2, p) for p in range(4)],
                8: [lambda p=p: _q_unit(0, 3, p) for p in range(4)],
                9: [lambda t=t, p=p: _op_unit(t, p) for t in (4, 5) for p in (0, 1)],
                10: [lambda t=t, p=p: _op_unit(t, p) for t in (6, 7) for p in (0, 1)],
                11: [lambda p=p: _q_unit(1, 3, p) for p in range(4)],
                13: [lambda t=t, p=p: _op_unit(t, p) for t in (8, 9) for p in (0, 1)],
                14: [
                    lambda t=t, p=p: _op_unit(t, p) for t in (10, 11) for p in (0, 1)
                ],
            }
            iters = [(qb, h) for qb in range(4) for h in range(HPG)]
            for i in range(1, 17):
                pv_i = i - 1  # PV lags the score stream by one extra step
                qbp, hp = iters[pv_i]
                units = fillers.get(i, [])
                for u in range(8):
                    _pv_unit(hp, qbp, u)
                    if u % 2 == 0 and u // 2 < len(units):
                        units[u // 2]()
                    if i + 1 < 16:
                        qb1, h1 = iters[i + 1]
                        _scores_unit(h1, qb1, u)
                    if u % 2 == 1 and 4 + u // 2 < len(units):
                        units[4 + u // 2]()
                if pv_i < 15:
                    _norm(hp, qbp)
            # tail: pipeline the last normalize with the last out-projection
            for ck in range(4):
                _norm_chunk(3, 3, ck, 128)
                _outproj([12 + ck])
            del yext_tiles[(3, 3)]

            if debug:
                dq = nc.dram_tensor("dbg_q", [128, 2, T], MMDT, kind="ExternalOutput")
                dk = nc.dram_tensor("dbg_k", [128, 2, T], MMDT, kind="ExternalOutput")
                dy = nc.dram_tensor("dbg_y", [128, 2, T], MMDT, kind="ExternalOutput")
                dv = nc.dram_tensor(
                    "dbg_v", [128, 16, HPG, 128], MMDT, kind="ExternalOutput"
                )
                nc.sync.dma_start(out=dq[:, :, :], in_=qt_sb[:])
                nc.sync.dma_start(out=dk[:, :, :], in_=kt_sb[:])
                nc.sync.dma_start(out=dy[:, :, :], in_=yt_sb[:])
                nc.sync.dma_start(out=dv[:, :, :, :], in_=v_sb[:])

    nc.finalize()
    return nc


def _get_nc():
    global _CACHED_NC
    if _CACHED_NC is None:
        _CACHED_NC = _build()
    return _CACHED_NC


def make_in_maps(x, Wq, bq, Wk, bk, Wv, Wo):
    """Per-core input dicts (host-side sharding + layout + fp16 cast)."""
    xts = [
        np.ascontiguousarray(np.asarray(x[b], np.float32).T).astype(NPDT)
        for b in range(B)
    ]
    wq_f = np.asarray(Wq, np.float32) / 8.0
    wk_f = np.asarray(Wk, np.float32)
    wv_f = np.asarray(Wv, np.float32)
    wo_f = np.asarray(Wo, np.float32)
    bq_f = np.asarray(bq, np.float32) / 8.0
    bk_f = np.asarray(bk, np.float32)
    in_maps = []
    for c in range(NCORES):
        b, g = c // GROUPS, c % GROUPS
        js = slice(g * JW, (g + 1) * JW)
        in_maps.append(
            {
                "xt": xts[b],
                "wq": np.ascontiguousarray(wq_f[:, js]).astype(NPDT),
                "wk": np.ascontiguousarray(wk_f[:, js]).astype(NPDT),
                "wv": np.ascontiguousarray(wv_f[:, js]).astype(NPDT),
                "wo": np.ascontiguousarray(wo_f[js, :]).astype(NPDT),
                "bq": np.ascontiguousarray(bq_f[js]),
                "bk": np.ascontiguousarray(bk_f[js]),
            }
        )
    return in_maps


def combine(results, bias_row):
    """Sum per-core head-group partials and add the host-side bias row."""
    out = np.zeros((B, T, C), np.float32)
    for c in range(NCORES):
        out[c // GROUPS] += results[c]["out"].astype(np.float32)
    out += bias_row
    return out


def kernel(x, Wq, bq, Wk, bk, Wv, bv, Wo, bo):
    nc = _get_nc()
    in_maps = make_in_maps(x, Wq, bq, Wk, bk, Wv, Wo)
    res = run_bass_kernel_spmd(nc, in_maps, core_ids=list(range(NCORES)))
    bias_row = (
        np.asarray(bv, np.float32) @ np.asarray(Wo, np.float32)
        + np.asarray(bo, np.float32)
    ).astype(np.float32)
    return combine(res.results, bias_row)
